# revision 4
# baseline (speedup 1.0000x reference)
"""Multi-headed attention kernel for 8 Trainium2 NeuronCores.

Problem: B=4, S=2048, E=1024, H=16, D=64 (torch-convention Linears, no bias
on q/k/v, bias on output projection).

Sharding: core c handles (batch b = c//2, query half sh = c%2).  Each core
computes Q for its 1024 query rows, K/V for the full 2048 keys of its batch
(duplicated across the pair of cores sharing a batch -- cheaper than any
cross-core collective at these sizes), all 16 heads of attention for its
rows, and the output projection + bias.  Zero collectives.

Layout (feature dim on partitions; scores computed transposed):
  qT[f, q]  = sum_e WqT[e, f] * XT[e, q]          kT[f, s] likewise
  V[s, f]   = sum_e XT[e, s-chunk] * WvT[e, f]    (natural layout)
  scoresT[k, q] = sum_d kT[h*64+d, kc] * qT[h*64+d, q]    (K=64 matmuls,
      head pair packed in complementary PE ROW groups -> concurrent)
  EX = exp(scoresT / 8)            (ACT engine, PSUM -> SBUF bf16)
  ctxT[hh*64+m, q] = sum_k V[k, h*64+m] * EX[k, q]   per head, M=64, the two
      heads packed in complementary PE COLUMN groups -> concurrent (2x over
      the old M=65 ones-column formulation)
  den[q] = sum_k EX[k, q]: EX tiles accumulated elementwise on DVE/GpSimd
      (alternating), then one gpsimd partition_all_reduce per segment
      (f32 internal); normalization = DVE multiply by reciprocal.
  out[s, e] = sum_f ctxT_norm[f, s-chunk] * WoT[f, e] + bo (bias added on
      DVE from a partition-broadcast bo tile; no PE bias matmuls)

Scheduling: fc-outer over the 8 head pairs, two 512-query segments each.
Projection work for pair fc+1 and V feature-halves is emitted as paired-
weight filler units (two PSUM accumulators sharing each LDWEIGHTS) inside
the 16-key-chunk slot loop, keeping the PE dense while ACT streams exps.
PSUM: scores 2x[128,1024] + ctx 1x[128,512] + filler 3x[128,512] = 8 banks.
"""

import os

import numpy as np
import ml_dtypes

import concourse.bass as bass
from concourse import bacc
from concourse import bass_isa
import concourse.mybir as mybir
import concourse.tile as tile
from concourse.bass_utils import run_bass_kernel_spmd

B, S, E, H = 4, 2048, 1024, 16
D = E // H  # 64
P = 128
SL = S // 2     # local query rows per core (1024)
NCORES = 8
EC = E // P     # 8 e-chunks
FC = E // P     # 8 feature chunks (head pairs)
SC = S // P     # 16 s-chunks (V natural layout)
KC = S // P     # 16 key chunks (scores partition dim)

F32 = mybir.dt.float32
BF16 = mybir.dt.bfloat16
EXPF = mybir.ActivationFunctionType.Exp
NPBF = ml_dtypes.bfloat16

_CACHE = {}


def build():
    nc = bacc.Bacc(
        "TRN2",
        target_bir_lowering=False,
        debug=False,
        num_devices=NCORES,
    )

    xt_d = nc.dram_tensor("xt", [E, S], BF16, kind="ExternalInput").ap()
    wqt_d = nc.dram_tensor("wqt", [E, E], BF16, kind="ExternalInput").ap()
    wkt_d = nc.dram_tensor("wkt", [E, E], BF16, kind="ExternalInput").ap()
    wvt_d = nc.dram_tensor("wvt", [E, E], BF16, kind="ExternalInput").ap()
    wot_d = nc.dram_tensor("wot", [E, E], BF16, kind="ExternalInput").ap()
    bo_d = nc.dram_tensor("bo", [1, E], BF16, kind="ExternalInput").ap()
    out_d = nc.dram_tensor("out", [SL, E], F32, kind="ExternalOutput").ap()

    with tile.TileContext(nc) as tc:
     with tc.tile_pool(name="persist", bufs=1) as persist:
        qt_sb = persist.tile([P, FC, SL], BF16, tag="qt")
        kt_sb = persist.tile([P, FC, S], BF16, tag="kt")
        v_sb = persist.tile([P, SC, E], BF16, tag="v")
        ctxt_sb = persist.tile([P, FC, SL], BF16, tag="ctxt")
        bo_row = persist.tile([1, E], BF16, tag="bo_row")
        bo_bc = persist.tile([P, E], BF16, tag="bo_bc")
        nc.sync.dma_start(out=bo_row[:], in_=bo_d[:])
        nc.gpsimd.partition_broadcast(bo_bc[:], bo_row[:], channels=P)

        with tc.tile_pool(name="expden", bufs=1) as expden:

            def new_ex():
                return expden.tile([P, 1024], BF16, tag="exp", bufs=8, name="ex")

            def new_den():
                return expden.tile([P, 1024], BF16, tag="den", bufs=2, name="den")

            def new_denall():
                return expden.tile([P, 1024], F32, tag="denall", bufs=2, name="da")

            def new_rec():
                return expden.tile([P, 1024], F32, tag="rec", bufs=2, name="rec")

            with tc.tile_pool(name="xwq", bufs=1) as xwq:
                # ---- input DMAs, ordered for startup latency ----
                w_slices = {}

                def load_wslices(fc):
                    """16 [128,128] lhsT slices (all e-chunks) of WqT/WkT."""
                    qs, ks = [], []
                    for w_dram, lst in ((wqt_d, qs), (wkt_d, ks)):
                        for ec in range(EC):
                            t = xwq.tile([P, P], BF16, tag="wqk", bufs=36,
                                         name="wqk")
                            nc.sync.dma_start(
                                out=t[:],
                                in_=w_dram[ec * P:(ec + 1) * P,
                                           fc * P:(fc + 1) * P],
                            )
                            lst.append(t)
                    w_slices[fc] = (qs, ks)

                load_wslices(0)
                xts = []  # xts[ec][half] = [P, 1024]
                for ec in range(EC):
                    row = []
                    for hx in range(2):
                        t = xwq.tile([P, 1024], BF16, tag="x", bufs=16,
                                     name="x")
                        nc.sync.dma_start(
                            out=t[:],
                            in_=xt_d[ec * P:(ec + 1) * P,
                                     hx * 1024:(hx + 1) * 1024],
                        )
                        row.append(t)
                    xts.append(row)
                wv = []
                for ec in range(EC):
                    t = xwq.tile([P, E], BF16, tag="wv", bufs=8, name="wv")
                    nc.sync.dma_start(out=t[:],
                                      in_=wvt_d[ec * P:(ec + 1) * P, :])
                    wv.append(t)

                # ---------- filler unit emitters (paired LDWEIGHTS) ----------
                def q_pair(pool, fc):
                    """qt[:, fc, :] (both 512-blocks), wq slice loaded once/ec."""
                    qs = w_slices[fc][0]
                    ps0 = pool.tile([P, 512], F32, tag="pj", name="pj")
                    ps1 = pool.tile([P, 512], F32, tag="pj", name="pj")
                    for ec in range(EC):
                        st, sp = ec == 0, ec == EC - 1
                        nc.tensor.matmul(ps0[:], qs[ec][:],
                                         xts[ec][0][:, 0:512], start=st, stop=sp)
                        nc.tensor.matmul(ps1[:], qs[ec][:],
                                         xts[ec][0][:, 512:1024], start=st, stop=sp)
                    nc.vector.tensor_copy(out=qt_sb[:, fc, 0:512], in_=ps0[:])
                    nc.vector.tensor_copy(out=qt_sb[:, fc, 512:1024], in_=ps1[:])

                def k_pair(pool, fc, j):
                    """kt[:, fc, j*1024:(j+1)*1024], wk slice loaded once/ec."""
                    ks = w_slices[fc][1]
                    ps0 = pool.tile([P, 512], F32, tag="pj", name="pj")
                    ps1 = pool.tile([P, 512], F32, tag="pj", name="pj")
                    for ec in range(EC):
                        st, sp = ec == 0, ec == EC - 1
                        nc.tensor.matmul(ps0[:], ks[ec][:],
                                         xts[ec][j][:, 0:512], start=st, stop=sp)
                        nc.tensor.matmul(ps1[:], ks[ec][:],
                                         xts[ec][j][:, 512:1024], start=st, stop=sp)
                    base = j * 1024
                    nc.vector.tensor_copy(out=kt_sb[:, fc, base:base + 512],
                                          in_=ps0[:])
                    nc.vector.tensor_copy(out=kt_sb[:, fc, base + 512:base + 1024],
                                          in_=ps1[:])

                def v_unit(pool, sc, fb, bufs=None):
                    """v_sb[:, sc, fb*512:...]: natural-layout V chunk."""
                    ps = pool.tile([P, 512], F32, tag="pj", bufs=bufs,
                                   name="pj")
                    for ec in range(EC):
                        nc.tensor.matmul(
                            ps[:],
                            xts[ec][sc // 8][:, (sc % 8) * P:(sc % 8 + 1) * P],
                            wv[ec][:, fb * 512:(fb + 1) * 512],
                            start=(ec == 0), stop=(ec == EC - 1),
                        )
                    nc.vector.tensor_copy(
                        out=v_sb[:, sc, fb * 512:(fb + 1) * 512], in_=ps[:])

                # ---------------- upfront: QK(fc0) + V(sc0-5, fb0) ----------------
                with tc.tile_pool(name="psum_u", bufs=6, space="PSUM") as psum_u:
                    psq = [psum_u.tile([P, 512], F32, tag="u", name="u")
                           for _ in range(2)]
                    psk = [psum_u.tile([P, 512], F32, tag="u", name="u")
                           for _ in range(4)]
                    qs0, ks0 = w_slices[0]
                    for ec in range(EC):
                        st, sp = ec == 0, ec == EC - 1
                        nc.tensor.matmul(psq[0][:], qs0[ec][:],
                                         xts[ec][0][:, 0:512], start=st, stop=sp)
                        nc.tensor.matmul(psq[1][:], qs0[ec][:],
                                         xts[ec][0][:, 512:1024], start=st, stop=sp)
                        for kb in range(4):
                            nc.tensor.matmul(
                                psk[kb][:], ks0[ec][:],
                                xts[ec][kb // 2][:, (kb % 2) * 512:(kb % 2) * 512 + 512],
                                start=st, stop=sp)
                    nc.vector.tensor_copy(out=qt_sb[:, 0, 0:512], in_=psq[0][:])
                    nc.vector.tensor_copy(out=qt_sb[:, 0, 512:1024], in_=psq[1][:])
                    for kb in range(4):
                        nc.vector.tensor_copy(
                            out=kt_sb[:, 0, kb * 512:(kb + 1) * 512],
                            in_=psk[kb][:])
                    for sc in range(6):
                        v_unit(psum_u, sc, 0, bufs=2)

                # ---------------- pair loop ----------------
                with (
                    tc.tile_pool(name="psum_sc", bufs=2, space="PSUM") as psum_sc,
                    tc.tile_pool(name="psum_cx", bufs=1, space="PSUM") as psum_cx,
                    tc.tile_pool(name="psum_pj", bufs=3, space="PSUM") as psum_pj,
                ):
                    # per-fc filler queues (closures); popped one per slot
                    queues = {fc: [] for fc in range(FC)}
                    queues[0] = (
                        [lambda sc=sc: v_unit(psum_pj, sc, 0) for sc in range(6, 16)]
                    )
                    vf1 = [lambda sc=sc: v_unit(psum_pj, sc, 1) for sc in range(16)]
                    vf1_share = {1: vf1[0:4], 2: vf1[4:9], 3: vf1[9:13],
                                 4: vf1[13:16]}
                    for fc in range(FC - 1):
                        nfc = fc + 1
                        queues[fc] += [
                            lambda f=nfc: q_pair(psum_pj, f),
                            lambda f=nfc: k_pair(psum_pj, f, 0),
                            lambda f=nfc: k_pair(psum_pj, f, 1),
                        ]
                    for fc, units in vf1_share.items():
                        queues[fc] += units

                    def segment(qv, fc):
                        hA, hB = 2 * fc, 2 * fc + 1
                        q = queues[fc]
                        ctx_ps = psum_cx.tile([P, 512], F32, tag="ctx",
                                              name="ctx")
                        den = new_den()
                        for kc in range(KC):
                            sc_ps = psum_sc.tile([P, 1024], F32, tag="sc",
                                                 name="sc")
                            for hh in (0, 1):
                                po = hh * D
                                nc.tensor.matmul(
                                    sc_ps[:, hh * 512:hh * 512 + 512],
                                    kt_sb[po:po + D, fc, kc * P:(kc + 1) * P],
                                    qt_sb[po:po + D, fc,
                                          qv * 512:qv * 512 + 512],
                                    start=True, stop=True)
                            ex = new_ex()
                            nc.scalar.activation(ex[:], sc_ps[:], EXPF,
                                                 scale=0.125)
                            for hh, h in ((0, hA), (1, hB)):
                                nc.tensor.matmul(
                                    ctx_ps[hh * D:(hh + 1) * D, :],
                                    v_sb[:, kc, h * D:(h + 1) * D],
                                    ex[:, hh * 512:hh * 512 + 512],
                                    start=(kc == 0), stop=(kc == KC - 1))
                            if kc == 0:
                                nc.vector.tensor_copy(out=den[:], in_=ex[:])
                            elif kc % 2:
                                nc.gpsimd.tensor_add(out=den[:], in0=den[:],
                                                     in1=ex[:])
                            else:
                                nc.vector.tensor_add(out=den[:], in0=den[:],
                                                     in1=ex[:])
                            if q and (fc == 0 or kc % 2):
                                q.pop(0)()
                        # drain: ctx -> sbuf, denominators -> reciprocal -> mul
                        dst = ctxt_sb[:, fc, qv * 512:qv * 512 + 512]
                        nc.vector.tensor_copy(out=dst, in_=ctx_ps[:])
                        da = new_denall()
                        nc.gpsimd.partition_all_reduce(
                            da[:], den[:], channels=P,
                            reduce_op=bass_isa.ReduceOp.add)
                        rec = new_rec()
                        nc.vector.reciprocal_approx_fast(out=rec[:], in_=da[:])
                        nc.vector.tensor_mul(out=dst[0:D, :], in0=dst[0:D, :],
                                             in1=rec[0:D, 0:512])
                        nc.vector.tensor_mul(out=dst[D:P, :], in0=dst[D:P, :],
                                             in1=rec[D:P, 512:1024])

                    for fc in range(FC):
                        if fc + 1 < FC:
                            load_wslices(fc + 1)
                        segment(0, fc)
                        segment(1, fc)
                        while queues[fc]:
                            queues[fc].pop(0)()

            # ---------------- output projection ----------------
            with (
                tc.tile_pool(name="wo", bufs=1) as wopool,
                tc.tile_pool(name="outp", bufs=3) as outpool,
                tc.tile_pool(name="psum_o", bufs=4, space="PSUM") as psum_o,
            ):
                wot_sb = wopool.tile([P, FC, E], BF16, tag="wot")
                for fcc in range(FC):
                    nc.sync.dma_start(
                        out=wot_sb[:, fcc, :],
                        in_=wot_d[fcc * P:(fcc + 1) * P, :])

                for sc in range(SL // P):
                    po0 = psum_o.tile([P, 512], F32, tag="po", name="po")
                    po1 = psum_o.tile([P, 512], F32, tag="po", name="po")
                    for fcc in range(FC):
                        st, sp = fcc == 0, fcc == FC - 1
                        lhs = ctxt_sb[:, fcc, sc * P:(sc + 1) * P]
                        nc.tensor.matmul(po0[:], lhs, wot_sb[:, fcc, 0:512],
                                         start=st, stop=sp)
                        nc.tensor.matmul(po1[:], lhs, wot_sb[:, fcc, 512:1024],
                                         start=st, stop=sp)
                    ot = outpool.tile([P, E], F32, tag="out", name="out")
                    nc.vector.tensor_add(out=ot[:, 0:512], in0=po0[:],
                                         in1=bo_bc[:, 0:512])
                    nc.vector.tensor_add(out=ot[:, 512:1024], in0=po1[:],
                                         in1=bo_bc[:, 512:1024])
                    nc.sync.dma_start(out=out_d[sc * P:(sc + 1) * P, :],
                                      in_=ot[:])

    nc.compile()
    return nc


def _prep_inputs(X, Wq, Wk, Wv, Wo, bo):
    X = np.asarray(X, dtype=np.float32)
    wqt = np.ascontiguousarray(np.asarray(Wq, np.float32).T).astype(NPBF)
    wkt = np.ascontiguousarray(np.asarray(Wk, np.float32).T).astype(NPBF)
    wvt = np.ascontiguousarray(np.asarray(Wv, np.float32).T).astype(NPBF)
    wot = np.ascontiguousarray(np.asarray(Wo, np.float32).T).astype(NPBF)
    bo2 = np.asarray(bo, np.float32).reshape(1, E).astype(NPBF)

    in_maps = []
    for c in range(NCORES):
        b, sh = c // 2, c % 2
        xt = np.ascontiguousarray(X[b].T)  # [E, S]
        if sh == 1:  # rotate so the local query half comes first
            xt = np.concatenate([xt[:, SL:], xt[:, :SL]], axis=1)
        in_maps.append(
            {
                "xt": np.ascontiguousarray(xt.astype(NPBF)),
                "wqt": wqt,
                "wkt": wkt,
                "wvt": wvt,
                "wot": wot,
                "bo": bo2,
            }
        )
    return in_maps


LAST_EXEC_NS = None
LAST_RESULTS = None


def _ensure_ntff_hook_importable():
    """bass_utils imports antenv.axon_hooks when tracing is requested (e.g.
    BASS_TRACE=1 in the environment).  The RL container's antenv stub lacks
    that module; register a no-op fallback so tracing degrades gracefully
    instead of crashing.  If a real antenv.axon_hooks exists, do nothing."""
    import sys
    import types

    try:
        import antenv.axon_hooks  # noqa: F401

        return
    except ImportError:
        pass
    try:
        import antenv

        mod = types.ModuleType("antenv.axon_hooks")
        _hook = [None]
        mod.set_axon_ntff_profile_hook = lambda h: _hook.__setitem__(0, h)
        mod.get_axon_ntff_profile_hook = lambda: _hook[0]
        sys.modules["antenv.axon_hooks"] = mod
        antenv.axon_hooks = mod
        try:
            from trn_agent_boot.trn_boot import _ntff_profile_via_ctypes

            mod.set_axon_ntff_profile_hook(
                _ntff_profile_via_ctypes("/opt/axon/libaxon_pjrt.so")
            )
        except Exception:
            pass
    except Exception:
        pass


def _run(in_maps, trace=False):
    global LAST_EXEC_NS, LAST_RESULTS
    _ensure_ntff_hook_importable()
    if "nc" not in _CACHE:
        _CACHE["nc"] = build()
    res = run_bass_kernel_spmd(
        _CACHE["nc"],
        in_maps,
        core_ids=list(range(NCORES)),
        trace=trace,
    )
    LAST_RESULTS = res
    LAST_EXEC_NS = res.exec_time_ns
    return res


def kernel(X, Wq, Wk, Wv, Wo, bo):
    in_maps = _prep_inputs(X, Wq, Wk, Wv, Wo, bo)
    res = _run(in_maps, trace=bool(int(os.environ.get("KERNEL_TRACE", "0"))))
    out = np.empty((B, S, E), np.float32)
    for c in range(NCORES):
        b, sh = c // 2, c % 2
        out[b, sh * SL : (sh + 1) * SL, :] = res.results[c]["out"]
    return out


# revision 6
# speedup vs baseline: 1.1538x; 1.1538x over previous
"""Multi-headed attention kernel for 8 Trainium2 NeuronCores.

Problem: B=4, S=2048, E=1024, H=16, D=64 (torch-convention Linears, no bias
on q/k/v, bias on output projection).

Sharding: core c handles (batch b = c//2, query half sh = c%2).  Each core
computes Q for its 1024 query rows, K/V for the full 2048 keys of its batch
(duplicated across the pair of cores sharing a batch -- cheaper than any
cross-core collective at these sizes), all 16 heads of attention for its
rows, and the output projection + bias.  Zero collectives.

Layout (feature dim on partitions; scores computed transposed):
  qT[f, q]  = sum_e WqT[e, f] * XT[e, q]          kT[f, s] likewise
  V[s, f]   = sum_e XT[e, s-chunk] * WvT[e, f]    (natural layout)
  scoresT[k, q] = sum_d kT[h*64+d, kc] * qT[h*64+d, q]    (K=64 matmuls,
      head pair packed in complementary PE ROW groups -> concurrent)
  EX = exp(scoresT / 8)            (ACT engine, PSUM -> SBUF bf16)
  ctxT[hh*64+m, q] = sum_k V[k, h*64+m] * EX[k, q]   per head, M=64, the two
      heads packed in complementary PE COLUMN groups -> concurrent (2x over
      an M=65 ones-column formulation)
  den[q] = sum_k EX[k, q]: EX tiles accumulated elementwise in TWO
      independent chains (DVE even kc, GpSimd odd kc; merged at drain), then
      one gpsimd partition_all_reduce per segment (f32 internal);
      normalization = DVE multiply by reciprocal.  Keeping the chains
      engine-local avoids cross-engine semaphore ping-pong that would gate
      the slot rate and let the PE HAM clock-gate re-throttle.
  out[s, e] = sum_f ctxT_norm[f, s-chunk] * WoT[f, e] + bo (bias added on
      DVE from a partition-broadcast bo tile; no PE bias matmuls)

Scheduling: fc-outer over the 8 head pairs, two 512-query segments each.
Projection work for pair fc+1 and V feature-halves is emitted as paired-
weight filler units (two PSUM accumulators sharing each LDWEIGHTS) inside
the 16-key-chunk slot loop, keeping the PE dense while ACT streams exps.
The first query-half's output projection runs as fillers inside the last
pair's second segment; only the second half's runs in the tail.
PSUM: scores 2x[128,1024] + ctx 1x[128,512] + filler 3x[128,512] = 8 banks.
"""

import os

import numpy as np
import ml_dtypes

import concourse.bass as bass
from concourse import bacc
from concourse import bass_isa
import concourse.mybir as mybir
import concourse.tile as tile
from concourse.bass_utils import run_bass_kernel_spmd

B, S, E, H = 4, 2048, 1024, 16
D = E // H  # 64
P = 128
SL = S // 2     # local query rows per core (1024)
NCORES = 8
EC = E // P     # 8 e-chunks
FC = E // P     # 8 feature chunks (head pairs)
SC = S // P     # 16 s-chunks (V natural layout)
KC = S // P     # 16 key chunks (scores partition dim)

F32 = mybir.dt.float32
BF16 = mybir.dt.bfloat16
EXPF = mybir.ActivationFunctionType.Exp
NPBF = ml_dtypes.bfloat16

_CACHE = {}


def build():
    nc = bacc.Bacc(
        "TRN2",
        target_bir_lowering=False,
        debug=False,
        num_devices=NCORES,
    )

    xt_d = nc.dram_tensor("xt", [E, S], BF16, kind="ExternalInput").ap()
    wqt_d = nc.dram_tensor("wqt", [E, E], BF16, kind="ExternalInput").ap()
    wkt_d = nc.dram_tensor("wkt", [E, E], BF16, kind="ExternalInput").ap()
    wvt_d = nc.dram_tensor("wvt", [E, E], BF16, kind="ExternalInput").ap()
    wot_d = nc.dram_tensor("wot", [E, E], BF16, kind="ExternalInput").ap()
    bo_d = nc.dram_tensor("bo", [1, E], BF16, kind="ExternalInput").ap()
    out_d = nc.dram_tensor("out", [SL, E], F32, kind="ExternalOutput").ap()

    with tile.TileContext(nc) as tc:
     with tc.tile_pool(name="persist", bufs=1) as persist:
        qt_sb = persist.tile([P, FC, SL], BF16, tag="qt")
        kt_sb = persist.tile([P, FC, S], BF16, tag="kt")
        v_sb = persist.tile([P, SC, E], BF16, tag="v")
        ctxt_sb = persist.tile([P, FC, SL], BF16, tag="ctxt")
        bo_row = persist.tile([1, E], BF16, tag="bo_row")
        bo_bc = persist.tile([P, E], BF16, tag="bo_bc")
        nc.sync.dma_start(out=bo_row[:], in_=bo_d[:])
        nc.gpsimd.partition_broadcast(bo_bc[:], bo_row[:], channels=P)

        with tc.tile_pool(name="expden", bufs=1) as expden:

            def new_ex():
                return expden.tile([P, 1024], BF16, tag="exp", bufs=7, name="ex")

            def new_den(tag):
                return expden.tile([P, 1024], BF16, tag=tag, bufs=2, name=tag)

            def new_denall():
                return expden.tile([P, 1024], F32, tag="denall", bufs=2, name="da")

            def new_rec():
                return expden.tile([P, 1024], F32, tag="rec", bufs=2, name="rec")

            # ---------------- loop PSUM pools (manual; PSUM-space LIFO) ----
            # opened after the upfront pool is released, below.

            with tc.tile_pool(name="xw", bufs=1) as xw:
                # ---- input DMAs, ordered for startup latency ----
                w_slices = {}

                def load_wslices(fc):
                    """16 [128,128] lhsT slices (all e-chunks) of WqT/WkT."""
                    qs, ks = [], []
                    for w_dram, lst in ((wqt_d, qs), (wkt_d, ks)):
                        for ec in range(EC):
                            t = xw.tile([P, P], BF16, tag="wqk", bufs=36,
                                        name="wqk")
                            nc.sync.dma_start(
                                out=t[:],
                                in_=w_dram[ec * P:(ec + 1) * P,
                                           fc * P:(fc + 1) * P],
                            )
                            lst.append(t)
                    w_slices[fc] = (qs, ks)

                load_wslices(0)
                xts = []  # xts[ec][half] = [P, 1024]
                for ec in range(EC):
                    row = []
                    for hx in range(2):
                        t = xw.tile([P, 1024], BF16, tag="x", bufs=16,
                                    name="x")
                        nc.sync.dma_start(
                            out=t[:],
                            in_=xt_d[ec * P:(ec + 1) * P,
                                     hx * 1024:(hx + 1) * 1024],
                        )
                        row.append(t)
                    xts.append(row)

                # ---------- filler unit emitters (paired LDWEIGHTS) ----------
                def q_pair(pool, fc):
                    """qt[:, fc, :] (both 512-blocks), wq slice loaded once/ec."""
                    qs = w_slices[fc][0]
                    ps0 = pool.tile([P, 512], F32, tag="pj", name="pj")
                    ps1 = pool.tile([P, 512], F32, tag="pj", name="pj")
                    for ec in range(EC):
                        st, sp = ec == 0, ec == EC - 1
                        nc.tensor.matmul(ps0[:], qs[ec][:],
                                         xts[ec][0][:, 0:512], start=st, stop=sp)
                        nc.tensor.matmul(ps1[:], qs[ec][:],
                                         xts[ec][0][:, 512:1024], start=st, stop=sp)
                    nc.vector.tensor_copy(out=qt_sb[:, fc, 0:512], in_=ps0[:])
                    nc.vector.tensor_copy(out=qt_sb[:, fc, 512:1024], in_=ps1[:])

                def k_pair(pool, fc, j):
                    """kt[:, fc, j*1024:(j+1)*1024], wk slice loaded once/ec."""
                    ks = w_slices[fc][1]
                    ps0 = pool.tile([P, 512], F32, tag="pj", name="pj")
                    ps1 = pool.tile([P, 512], F32, tag="pj", name="pj")
                    for ec in range(EC):
                        st, sp = ec == 0, ec == EC - 1
                        nc.tensor.matmul(ps0[:], ks[ec][:],
                                         xts[ec][j][:, 0:512], start=st, stop=sp)
                        nc.tensor.matmul(ps1[:], ks[ec][:],
                                         xts[ec][j][:, 512:1024], start=st, stop=sp)
                    base = j * 1024
                    nc.vector.tensor_copy(out=kt_sb[:, fc, base:base + 512],
                                          in_=ps0[:])
                    nc.vector.tensor_copy(out=kt_sb[:, fc, base + 512:base + 1024],
                                          in_=ps1[:])

                queues = {fc: [] for fc in range(FC)}

                def segment(qv, fc, every_slot=False):
                    hA, hB = 2 * fc, 2 * fc + 1
                    q = queues[fc]
                    ctx_ps = psum_cx.tile([P, 512], F32, tag="ctx", name="ctx")
                    den_v = new_den("denv")
                    den_g = new_den("deng")
                    for kc in range(KC):
                        sc_ps = psum_sc.tile([P, 1024], F32, tag="sc",
                                             name="sc")
                        for hh in (0, 1):
                            po = hh * D
                            nc.tensor.matmul(
                                sc_ps[:, hh * 512:hh * 512 + 512],
                                kt_sb[po:po + D, fc, kc * P:(kc + 1) * P],
                                qt_sb[po:po + D, fc,
                                      qv * 512:qv * 512 + 512],
                                start=True, stop=True)
                        ex = new_ex()
                        nc.scalar.activation(ex[:], sc_ps[:], EXPF,
                                             scale=0.125)
                        for hh, h in ((0, hA), (1, hB)):
                            nc.tensor.matmul(
                                ctx_ps[hh * D:(hh + 1) * D, :],
                                v_sb[:, kc, h * D:(h + 1) * D],
                                ex[:, hh * 512:hh * 512 + 512],
                                start=(kc == 0), stop=(kc == KC - 1))
                        if kc == 0:
                            nc.vector.tensor_copy(out=den_v[:], in_=ex[:])
                        elif kc == 1:
                            nc.gpsimd.tensor_copy(out=den_g[:], in_=ex[:])
                        elif kc % 2 == 0 or kc == KC - 1:
                            nc.vector.tensor_add(out=den_v[:], in0=den_v[:],
                                                 in1=ex[:])
                        else:
                            nc.gpsimd.tensor_add(out=den_g[:], in0=den_g[:],
                                                 in1=ex[:])
                        if q and (every_slot or kc % 2):
                            q.pop(0)()
                    # drain: ctx -> sbuf; merge den chains -> PAR -> recip -> mul
                    dst = ctxt_sb[:, fc, qv * 512:qv * 512 + 512]
                    nc.vector.tensor_copy(out=dst, in_=ctx_ps[:])
                    nc.vector.tensor_add(out=den_v[:], in0=den_v[:],
                                         in1=den_g[:])
                    da = new_denall()
                    nc.gpsimd.partition_all_reduce(
                        da[:], den_v[:], channels=P,
                        reduce_op=bass_isa.ReduceOp.add)
                    rec = new_rec()
                    nc.vector.reciprocal_approx_fast(out=rec[:], in_=da[:])
                    nc.vector.tensor_mul(out=dst[0:D, :], in0=dst[0:D, :],
                                         in1=rec[0:D, 0:512])
                    nc.vector.tensor_mul(out=dst[D:P, :], in0=dst[D:P, :],
                                         in1=rec[D:P, 512:1024])

                with tc.tile_pool(name="wvp", bufs=1) as wvp:
                    wv = []
                    for ec in range(EC):
                        t = wvp.tile([P, E], BF16, tag="wv", bufs=8, name="wv")
                        nc.sync.dma_start(out=t[:],
                                          in_=wvt_d[ec * P:(ec + 1) * P, :])
                        wv.append(t)

                    def v_unit(pool, sc, fb, bufs=None):
                        """v_sb[:, sc, fb*512:...]: natural-layout V chunk."""
                        ps = pool.tile([P, 512], F32, tag="pj", bufs=bufs,
                                       name="pj")
                        for ec in range(EC):
                            nc.tensor.matmul(
                                ps[:],
                                xts[ec][sc // 8][:, (sc % 8) * P:(sc % 8 + 1) * P],
                                wv[ec][:, fb * 512:(fb + 1) * 512],
                                start=(ec == 0), stop=(ec == EC - 1),
                            )
                        nc.vector.tensor_copy(
                            out=v_sb[:, sc, fb * 512:(fb + 1) * 512], in_=ps[:])

                    # ---------------- upfront: QK(fc0) + V(sc0-5, fb0) -------
                    psum_u = tc.alloc_tile_pool(name="psum_u", bufs=6,
                                                space="PSUM")
                    psq = [psum_u.tile([P, 512], F32, tag="u", name="u")
                           for _ in range(2)]
                    psk = [psum_u.tile([P, 512], F32, tag="u", name="u")
                           for _ in range(4)]
                    qs0, ks0 = w_slices[0]
                    for ec in range(EC):
                        st, sp = ec == 0, ec == EC - 1
                        nc.tensor.matmul(psq[0][:], qs0[ec][:],
                                         xts[ec][0][:, 0:512], start=st, stop=sp)
                        nc.tensor.matmul(psq[1][:], qs0[ec][:],
                                         xts[ec][0][:, 512:1024], start=st, stop=sp)
                        for kb in range(4):
                            nc.tensor.matmul(
                                psk[kb][:], ks0[ec][:],
                                xts[ec][kb // 2][:, (kb % 2) * 512:(kb % 2) * 512 + 512],
                                start=st, stop=sp)
                    nc.vector.tensor_copy(out=qt_sb[:, 0, 0:512], in_=psq[0][:])
                    nc.vector.tensor_copy(out=qt_sb[:, 0, 512:1024], in_=psq[1][:])
                    for kb in range(4):
                        nc.vector.tensor_copy(
                            out=kt_sb[:, 0, kb * 512:(kb + 1) * 512],
                            in_=psk[kb][:])
                    for sc in range(6):
                        v_unit(psum_u, sc, 0, bufs=2)
                    psum_u.release()

                    # ---------------- pair loop (fc0-4 with wv live) --------
                    psum_sc = tc.alloc_tile_pool(name="psum_sc", bufs=2,
                                                 space="PSUM")
                    psum_cx = tc.alloc_tile_pool(name="psum_cx", bufs=1,
                                                 space="PSUM")
                    psum_pj = tc.alloc_tile_pool(name="psum_pj", bufs=3,
                                                 space="PSUM")

                    queues[0] = (
                        [lambda sc=sc: v_unit(psum_pj, sc, 0)
                         for sc in range(6, 16)]
                    )
                    vf1 = [lambda sc=sc: v_unit(psum_pj, sc, 1)
                           for sc in range(16)]
                    vf1_share = {1: vf1[0:2], 2: vf1[2:7], 3: vf1[7:12],
                                 4: vf1[12:16]}
                    for fc in range(FC - 1):
                        nfc = fc + 1
                        queues[fc] += [
                            lambda f=nfc: q_pair(psum_pj, f),
                            lambda f=nfc: k_pair(psum_pj, f, 0),
                            lambda f=nfc: k_pair(psum_pj, f, 1),
                        ]
                    for fc, units in vf1_share.items():
                        queues[fc] += units

                    for fc in range(5):
                        if fc + 1 < FC:
                            load_wslices(fc + 1)
                        segment(0, fc, every_slot=(fc == 0))
                        segment(1, fc, every_slot=(fc == 0))
                        while queues[fc]:
                            queues[fc].pop(0)()
                # wv released; fc5-6 (x + wqk still live)
                for fc in (5, 6):
                    if fc + 1 < FC:
                        load_wslices(fc + 1)
                    segment(0, fc)
                    segment(1, fc)
                    while queues[fc]:
                        queues[fc].pop(0)()

            # xw released: fc7 + output projection
            with (
                tc.tile_pool(name="wo", bufs=1) as wopool,
                tc.tile_pool(name="outp", bufs=3) as outpool,
            ):
                wot_sb = wopool.tile([P, FC, E], BF16, tag="wot")
                for fcc in range(FC):
                    nc.sync.dma_start(
                        out=wot_sb[:, fcc, :],
                        in_=wot_d[fcc * P:(fcc + 1) * P, :])

                ots = {}

                def o_half(sc, eb):
                    po = psum_pj.tile([P, 512], F32, tag="pj", name="pj")
                    for fcc in range(FC):
                        st, sp = fcc == 0, fcc == FC - 1
                        nc.tensor.matmul(po[:],
                                         ctxt_sb[:, fcc, sc * P:(sc + 1) * P],
                                         wot_sb[:, fcc,
                                                eb * 512:(eb + 1) * 512],
                                         start=st, stop=sp)
                    if eb == 0:
                        ots[sc] = outpool.tile([P, E], F32, tag="out",
                                               name="out")
                    ot = ots[sc]
                    nc.vector.tensor_add(out=ot[:, eb * 512:(eb + 1) * 512],
                                         in0=po[:],
                                         in1=bo_bc[:, eb * 512:(eb + 1) * 512])
                    if eb == 1:
                        nc.sync.dma_start(out=out_d[sc * P:(sc + 1) * P, :],
                                          in_=ot[:])

                segment(0, 7)
                queues[7] = [lambda sc=sc, eb=eb: o_half(sc, eb)
                             for sc in range(4) for eb in range(2)]
                segment(1, 7)
                while queues[7]:
                    queues[7].pop(0)()
                for sc in range(4, SL // P):
                    o_half(sc, 0)
                    o_half(sc, 1)

                psum_pj.release()
                psum_cx.release()
                psum_sc.release()

    nc.compile()
    return nc


def _prep_inputs(X, Wq, Wk, Wv, Wo, bo):
    X = np.asarray(X, dtype=np.float32)
    wqt = np.ascontiguousarray(np.asarray(Wq, np.float32).T).astype(NPBF)
    wkt = np.ascontiguousarray(np.asarray(Wk, np.float32).T).astype(NPBF)
    wvt = np.ascontiguousarray(np.asarray(Wv, np.float32).T).astype(NPBF)
    wot = np.ascontiguousarray(np.asarray(Wo, np.float32).T).astype(NPBF)
    bo2 = np.asarray(bo, np.float32).reshape(1, E).astype(NPBF)

    in_maps = []
    for c in range(NCORES):
        b, sh = c // 2, c % 2
        xt = np.ascontiguousarray(X[b].T)  # [E, S]
        if sh == 1:  # rotate so the local query half comes first
            xt = np.concatenate([xt[:, SL:], xt[:, :SL]], axis=1)
        in_maps.append(
            {
                "xt": np.ascontiguousarray(xt.astype(NPBF)),
                "wqt": wqt,
                "wkt": wkt,
                "wvt": wvt,
                "wot": wot,
                "bo": bo2,
            }
        )
    return in_maps


LAST_EXEC_NS = None
LAST_RESULTS = None


def _ensure_ntff_hook_importable():
    """bass_utils imports antenv.axon_hooks when tracing is requested (e.g.
    BASS_TRACE=1 in the environment).  The RL container's antenv stub lacks
    that module; register a no-op fallback so tracing degrades gracefully
    instead of crashing.  If a real antenv.axon_hooks exists, do nothing."""
    import sys
    import types

    try:
        import antenv.axon_hooks  # noqa: F401

        return
    except ImportError:
        pass
    try:
        import antenv

        mod = types.ModuleType("antenv.axon_hooks")
        _hook = [None]
        mod.set_axon_ntff_profile_hook = lambda h: _hook.__setitem__(0, h)
        mod.get_axon_ntff_profile_hook = lambda: _hook[0]
        sys.modules["antenv.axon_hooks"] = mod
        antenv.axon_hooks = mod
        try:
            from trn_agent_boot.trn_boot import _ntff_profile_via_ctypes

            mod.set_axon_ntff_profile_hook(
                _ntff_profile_via_ctypes("/opt/axon/libaxon_pjrt.so")
            )
        except Exception:
            pass
    except Exception:
        pass


def _run(in_maps, trace=False):
    global LAST_EXEC_NS, LAST_RESULTS
    _ensure_ntff_hook_importable()
    if "nc" not in _CACHE:
        _CACHE["nc"] = build()
    res = run_bass_kernel_spmd(
        _CACHE["nc"],
        in_maps,
        core_ids=list(range(NCORES)),
        trace=trace,
    )
    LAST_RESULTS = res
    LAST_EXEC_NS = res.exec_time_ns
    return res


def kernel(X, Wq, Wk, Wv, Wo, bo):
    in_maps = _prep_inputs(X, Wq, Wk, Wv, Wo, bo)
    res = _run(in_maps, trace=bool(int(os.environ.get("KERNEL_TRACE", "0"))))
    out = np.empty((B, S, E), np.float32)
    for c in range(NCORES):
        b, sh = c // 2, c % 2
        out[b, sh * SL : (sh + 1) * SL, :] = res.results[c]["out"]
    return out


# revision 11
# speedup vs baseline: 1.4833x; 1.2856x over previous
"""Multi-headed attention kernel for 8 Trainium2 NeuronCores.

Problem: B=4, S=2048, E=1024, H=16, D=64 (torch-convention Linears, no bias
on q/k/v, bias on output projection).

Sharding: core c handles (batch b = c//2, query half sh = c%2).  Each core
computes Q for its 1024 query rows, K/V for the full 2048 keys of its batch
(duplicated across the pair of cores sharing a batch -- cheaper than any
cross-core collective at these sizes), all 16 heads of attention for its
rows, and the output projection + bias.  Zero collectives.

Layout (feature dim on partitions; scores computed transposed):
  qT[f, q]  = sum_e WqT[e, f] * XT[e, q]          kT[f, s] likewise
  V[s, f]   = sum_e XT[e, s-chunk] * WvT[e, f]    (natural layout)
  scoresT[k, q] = sum_d kT[h*64+d, kc] * qT[h*64+d, q]    (K=64 matmuls,
      head pair packed in complementary PE ROW groups -> concurrent)
  EX = exp(scoresT / 8)            (ACT engine, PSUM -> SBUF bf16)
  ctxT[hh*64+m, q] = sum_k V[k, h*64+m] * EX[k, q]   per head, M=64, the two
      heads packed in complementary PE COLUMN groups -> concurrent (2x over
      an M=65 ones-column formulation)
  den[q] = sum_k EX[k, q]: EX tiles accumulated elementwise in TWO
      independent chains (DVE even kc, GpSimd odd kc; merged at drain), then
      one gpsimd partition_all_reduce per segment (f32 internal);
      normalization = DVE multiply by reciprocal.  Keeping the chains
      engine-local avoids cross-engine semaphore ping-pong that would gate
      the slot rate and let the PE HAM clock-gate re-throttle.
  out[s, e] = sum_f ctxT_norm[f, s-chunk] * WoT[f, e] + bo (bias added on
      DVE from a partition-broadcast bo tile; no PE bias matmuls)

Scheduling: fc-outer over the 8 head pairs, two 512-query segments each.
Projection work for pair fc+1 and V feature-halves is emitted as paired-
weight filler units (two PSUM accumulators sharing each LDWEIGHTS) inside
the 16-key-chunk slot loop, keeping the PE dense while ACT streams exps.
The first query-half's output projection runs as fillers inside the last
pair's second segment; only the second half's runs in the tail.
PSUM: scores 2x[128,1024] + ctx 1x[128,512] + filler 3x[128,512] = 8 banks.
"""

import os

import numpy as np
import ml_dtypes

import concourse.bass as bass
from concourse import bacc
from concourse import bass_isa
import concourse.mybir as mybir
import concourse.tile as tile
from concourse.bass_utils import run_bass_kernel_spmd

B, S, E, H = 4, 2048, 1024, 16
D = E // H  # 64
P = 128
SL = S // 2     # local query rows per core (1024)
NCORES = 8
EC = E // P     # 8 e-chunks
FC = E // P     # 8 feature chunks (head pairs)
SC = S // P     # 16 s-chunks (V natural layout)
KC = S // P     # 16 key chunks (scores partition dim)

F32 = mybir.dt.float32
BF16 = mybir.dt.bfloat16
EXPF = mybir.ActivationFunctionType.Exp
NPBF = ml_dtypes.bfloat16

_CACHE = {}


def build():
    nc = bacc.Bacc(
        "TRN2",
        target_bir_lowering=False,
        debug=False,
        num_devices=NCORES,
    )

    xt_d = nc.dram_tensor("xt", [E, S], BF16, kind="ExternalInput").ap()
    wqt_d = nc.dram_tensor("wqt", [E, E], BF16, kind="ExternalInput").ap()
    wkt_d = nc.dram_tensor("wkt", [E, E], BF16, kind="ExternalInput").ap()
    wvt_d = nc.dram_tensor("wvt", [E, E], BF16, kind="ExternalInput").ap()
    wot_d = nc.dram_tensor("wot", [E, E], BF16, kind="ExternalInput").ap()
    bo_d = nc.dram_tensor("bo", [1, E], BF16, kind="ExternalInput").ap()
    out_d = nc.dram_tensor("out", [SL, E], F32, kind="ExternalOutput").ap()

    with tile.TileContext(nc) as tc:
     with tc.tile_pool(name="persist", bufs=1) as persist:
        qt_sb = persist.tile([P, FC, SL], BF16, tag="qt")
        kt_sb = persist.tile([P, FC, S], BF16, tag="kt")
        v_sb = persist.tile([P, SC, E], BF16, tag="v")
        ctxt_sb = persist.tile([P, FC, SL], BF16, tag="ctxt")
        bo_row = persist.tile([1, E], BF16, tag="bo_row")
        bo_bc = persist.tile([P, E], BF16, tag="bo_bc")
        ones_sb = persist.tile([P, P], BF16, tag="ones")
        nc.sync.dma_start(out=bo_row[:], in_=bo_d[:])
        nc.gpsimd.partition_broadcast(bo_bc[:], bo_row[:], channels=P)
        nc.vector.memset(ones_sb[:], 1.0)

        with tc.tile_pool(name="expden", bufs=1) as expden:

            def new_ex():
                return expden.tile([P, 1024], BF16, tag="exp", bufs=8, name="ex")

            def new_den(tag):
                return expden.tile([P, 1024], BF16, tag=tag, bufs=2, name=tag)

            def new_rec():
                return expden.tile([P, 1024], F32, tag="rec", bufs=2, name="rec")

            # ---------------- loop PSUM pools (manual; PSUM-space LIFO) ----
            # opened after the upfront pool is released, below.

            with tc.tile_pool(name="xw", bufs=1) as xw:
                # ---- input DMAs, ordered for startup latency ----
                w_slices = {}

                def load_wslices(fc):
                    """16 [128,128] lhsT slices (all e-chunks) of WqT/WkT."""
                    qs, ks = [], []
                    for w_dram, lst in ((wqt_d, qs), (wkt_d, ks)):
                        for ec in range(EC):
                            t = xw.tile([P, P], BF16, tag="wqk", bufs=36,
                                        name="wqk")
                            nc.sync.dma_start(
                                out=t[:],
                                in_=w_dram[ec * P:(ec + 1) * P,
                                           fc * P:(fc + 1) * P],
                            )
                            lst.append(t)
                    w_slices[fc] = (qs, ks)

                load_wslices(0)
                xts = []  # xts[ec][half] = [P, 1024]
                for ec in range(EC):
                    row = []
                    for hx in range(2):
                        t = xw.tile([P, 1024], BF16, tag="x", bufs=16,
                                    name="x")
                        nc.sync.dma_start(
                            out=t[:],
                            in_=xt_d[ec * P:(ec + 1) * P,
                                     hx * 1024:(hx + 1) * 1024],
                        )
                        row.append(t)
                    xts.append(row)

                # ---------- filler unit emitters (paired LDWEIGHTS) ----------
                def q_pair(pool, fc):
                    """qt[:, fc, :] (both 512-blocks), wq slice loaded once/ec."""
                    qs = w_slices[fc][0]
                    ps0 = pool.tile([P, 512], F32, tag="pj", name="pj")
                    ps1 = pool.tile([P, 512], F32, tag="pj", name="pj")
                    for ec in range(EC):
                        st, sp = ec == 0, ec == EC - 1
                        nc.tensor.matmul(ps0[:], qs[ec][:],
                                         xts[ec][0][:, 0:512], start=st, stop=sp)
                        nc.tensor.matmul(ps1[:], qs[ec][:],
                                         xts[ec][0][:, 512:1024], start=st, stop=sp)
                    nc.vector.tensor_copy(out=qt_sb[:, fc, 0:512], in_=ps0[:])
                    nc.vector.tensor_copy(out=qt_sb[:, fc, 512:1024], in_=ps1[:])

                def k_pair(pool, fc, j):
                    """kt[:, fc, j*1024:(j+1)*1024], wk slice loaded once/ec."""
                    ks = w_slices[fc][1]
                    ps0 = pool.tile([P, 512], F32, tag="pj", name="pj")
                    ps1 = pool.tile([P, 512], F32, tag="pj", name="pj")
                    for ec in range(EC):
                        st, sp = ec == 0, ec == EC - 1
                        nc.tensor.matmul(ps0[:], ks[ec][:],
                                         xts[ec][j][:, 0:512], start=st, stop=sp)
                        nc.tensor.matmul(ps1[:], ks[ec][:],
                                         xts[ec][j][:, 512:1024], start=st, stop=sp)
                    base = j * 1024
                    nc.vector.tensor_copy(out=kt_sb[:, fc, base:base + 512],
                                          in_=ps0[:])
                    nc.vector.tensor_copy(out=kt_sb[:, fc, base + 512:base + 1024],
                                          in_=ps1[:])

                queues = {fc: [] for fc in range(FC)}
                pending = []  # deferred norm finishers (popped next segment)

                def segment(qv, fc, every_slot=False, pop_from=1):
                    hA, hB = 2 * fc, 2 * fc + 1
                    q = queues[fc]
                    ctx_ps = psum_cx.tile([P, 512], F32, tag="ctx", name="ctx")
                    den_v = new_den("denv")
                    den_g = new_den("deng")
                    for kc in range(KC):
                        sc_ps = psum_sc.tile([P, 1024], F32, tag="sc",
                                             name="sc")
                        for hh in (0, 1):
                            po = hh * D
                            nc.tensor.matmul(
                                sc_ps[:, hh * 512:hh * 512 + 512],
                                kt_sb[po:po + D, fc, kc * P:(kc + 1) * P],
                                qt_sb[po:po + D, fc,
                                      qv * 512:qv * 512 + 512],
                                start=True, stop=True)
                        ex = new_ex()
                        nc.scalar.activation(ex[:], sc_ps[:], EXPF,
                                             scale=0.125)
                        for hh, h in ((0, hA), (1, hB)):
                            nc.tensor.matmul(
                                ctx_ps[hh * D:(hh + 1) * D, :],
                                v_sb[:, kc, h * D:(h + 1) * D],
                                ex[:, hh * 512:hh * 512 + 512],
                                start=(kc == 0), stop=(kc == KC - 1))
                        # two engine-local accumulation chains (merged at
                        # drain): cross-engine ping-pong would gate the slots
                        if kc == 0:
                            nc.vector.tensor_copy(out=den_v[:], in_=ex[:])
                        elif kc == 1:
                            nc.gpsimd.tensor_copy(out=den_g[:], in_=ex[:])
                        elif kc % 2 == 0:
                            nc.vector.tensor_add(out=den_v[:], in0=den_v[:],
                                                 in1=ex[:])
                        else:
                            nc.gpsimd.tensor_add(out=den_g[:], in0=den_g[:],
                                                 in1=ex[:])
                        if kc == 2 and pending:
                            pending.pop(0)()
                        if q and (every_slot or (kc % 2 and kc >= pop_from)):
                            q.pop(0)()
                    # drain: ctx -> sbuf; merge den chains (gpsimd)
                    dst = ctxt_sb[:, fc, qv * 512:qv * 512 + 512]
                    nc.vector.tensor_copy(out=dst, in_=ctx_ps[:])
                    nc.gpsimd.tensor_add(out=den_v[:], in0=den_v[:],
                                         in1=den_g[:])

                    def _finish(den_v=den_v, dst=dst):
                        # reduce+broadcast den over partitions via a ones
                        # matmul (PE), then reciprocal (DVE, PSUM read) and
                        # normalize ctxt in place.
                        po_a = psum_pj.tile([P, 512], F32, tag="pj", name="pj")
                        po_b = psum_pj.tile([P, 512], F32, tag="pj", name="pj")
                        nc.tensor.matmul(po_a[:], ones_sb[:], den_v[:, 0:512],
                                         start=True, stop=True)
                        nc.tensor.matmul(po_b[:], ones_sb[:],
                                         den_v[:, 512:1024],
                                         start=True, stop=True)
                        rec = new_rec()
                        nc.vector.reciprocal_approx_fast(out=rec[:, 0:512],
                                                         in_=po_a[:])
                        nc.vector.reciprocal_approx_fast(out=rec[:, 512:1024],
                                                         in_=po_b[:])
                        nc.vector.tensor_mul(out=dst[0:D, :], in0=dst[0:D, :],
                                             in1=rec[0:D, 0:512])
                        nc.vector.tensor_mul(out=dst[D:P, :], in0=dst[D:P, :],
                                             in1=rec[D:P, 512:1024])

                    pending.append(_finish)

                with tc.tile_pool(name="wvp", bufs=1) as wvp:
                    wv = []
                    for ec in range(EC):
                        t = wvp.tile([P, E], BF16, tag="wv", bufs=8, name="wv")
                        nc.sync.dma_start(out=t[:],
                                          in_=wvt_d[ec * P:(ec + 1) * P, :])
                        wv.append(t)

                    def v_unit(pool, sc, fb, bufs=None):
                        """v_sb[:, sc, fb*512:...]: natural-layout V chunk."""
                        ps = pool.tile([P, 512], F32, tag="pj", bufs=bufs,
                                       name="pj")
                        for ec in range(EC):
                            nc.tensor.matmul(
                                ps[:],
                                xts[ec][sc // 8][:, (sc % 8) * P:(sc % 8 + 1) * P],
                                wv[ec][:, fb * 512:(fb + 1) * 512],
                                start=(ec == 0), stop=(ec == EC - 1),
                            )
                        nc.vector.tensor_copy(
                            out=v_sb[:, sc, fb * 512:(fb + 1) * 512], in_=ps[:])

                    # ---------------- upfront: QK(fc0) + V(sc0-5, fb0) -------
                    psum_u = tc.alloc_tile_pool(name="psum_u", bufs=6,
                                                space="PSUM")
                    psq = [psum_u.tile([P, 512], F32, tag="u", name="u")
                           for _ in range(2)]
                    psk = [psum_u.tile([P, 512], F32, tag="u", name="u")
                           for _ in range(4)]
                    qs0, ks0 = w_slices[0]
                    for ec in range(EC):
                        st, sp = ec == 0, ec == EC - 1
                        nc.tensor.matmul(psq[0][:], qs0[ec][:],
                                         xts[ec][0][:, 0:512], start=st, stop=sp)
                        nc.tensor.matmul(psq[1][:], qs0[ec][:],
                                         xts[ec][0][:, 512:1024], start=st, stop=sp)
                        for kb in range(4):
                            nc.tensor.matmul(
                                psk[kb][:], ks0[ec][:],
                                xts[ec][kb // 2][:, (kb % 2) * 512:(kb % 2) * 512 + 512],
                                start=st, stop=sp)
                    nc.vector.tensor_copy(out=qt_sb[:, 0, 0:512], in_=psq[0][:])
                    nc.vector.tensor_copy(out=qt_sb[:, 0, 512:1024], in_=psq[1][:])
                    for kb in range(4):
                        nc.vector.tensor_copy(
                            out=kt_sb[:, 0, kb * 512:(kb + 1) * 512],
                            in_=psk[kb][:])
                    for sc in range(6):
                        v_unit(psum_u, sc, 0, bufs=2)
                    psum_u.release()

                    # ---------------- pair loop (fc0-4 with wv live) --------
                    psum_sc = tc.alloc_tile_pool(name="psum_sc", bufs=2,
                                                 space="PSUM")
                    psum_cx = tc.alloc_tile_pool(name="psum_cx", bufs=1,
                                                 space="PSUM")
                    psum_pj = tc.alloc_tile_pool(name="psum_pj", bufs=3,
                                                 space="PSUM")

                    queues[0] = (
                        [lambda sc=sc: v_unit(psum_pj, sc, 0)
                         for sc in range(6, 16)]
                    )
                    vf1 = [lambda sc=sc: v_unit(psum_pj, sc, 1)
                           for sc in range(16)]
                    vf1_share = {1: vf1[0:2], 2: vf1[2:7], 3: vf1[7:12],
                                 4: vf1[12:16]}
                    for fc in range(FC - 1):
                        nfc = fc + 1
                        queues[fc] += [
                            lambda f=nfc: q_pair(psum_pj, f),
                            lambda f=nfc: k_pair(psum_pj, f, 0),
                            lambda f=nfc: k_pair(psum_pj, f, 1),
                        ]
                    for fc, units in vf1_share.items():
                        queues[fc] += units

                    for fc in range(5):
                        if fc + 1 < FC:
                            load_wslices(fc + 1)
                        segment(0, fc, every_slot=(fc == 0))
                        segment(1, fc, every_slot=(fc == 0))
                        while queues[fc]:
                            queues[fc].pop(0)()
                # wv released; fc5-6 (x + wqk still live)
                for fc in (5, 6):
                    if fc + 1 < FC:
                        load_wslices(fc + 1)
                    segment(0, fc)
                    segment(1, fc)
                    while queues[fc]:
                        queues[fc].pop(0)()

            # xw released: fc7 + output projection
            with (
                tc.tile_pool(name="wo", bufs=1) as wopool,
                tc.tile_pool(name="outp", bufs=3) as outpool,
            ):
                wot_sb = wopool.tile([P, FC, E], BF16, tag="wot")
                for fcc in range(FC):
                    nc.sync.dma_start(
                        out=wot_sb[:, fcc, :],
                        in_=wot_d[fcc * P:(fcc + 1) * P, :])

                ots = {}

                def o_half(sc, eb):
                    po = psum_pj.tile([P, 512], F32, tag="pj", name="pj")
                    for fcc in range(FC):
                        st, sp = fcc == 0, fcc == FC - 1
                        nc.tensor.matmul(po[:],
                                         ctxt_sb[:, fcc, sc * P:(sc + 1) * P],
                                         wot_sb[:, fcc,
                                                eb * 512:(eb + 1) * 512],
                                         start=st, stop=sp)
                    if eb == 0:
                        ots[sc] = outpool.tile([P, E], F32, tag="out",
                                               name="out")
                    ot = ots[sc]
                    nc.vector.tensor_add(out=ot[:, eb * 512:(eb + 1) * 512],
                                         in0=po[:],
                                         in1=bo_bc[:, eb * 512:(eb + 1) * 512])
                    if eb == 1:
                        nc.sync.dma_start(out=out_d[sc * P:(sc + 1) * P, :],
                                          in_=ot[:])

                segment(0, 7)
                queues[7] = [lambda sc=sc, eb=eb: o_half(sc, eb)
                             for sc in range(4) for eb in range(2)]
                segment(1, 7, pop_from=3)
                while pending:
                    pending.pop(0)()
                while queues[7]:
                    queues[7].pop(0)()
                for sc in range(4, SL // P):
                    o_half(sc, 0)
                    o_half(sc, 1)

                psum_pj.release()
                psum_cx.release()
                psum_sc.release()

    nc.compile()
    return nc


def _prep_inputs(X, Wq, Wk, Wv, Wo, bo):
    X = np.asarray(X, dtype=np.float32)
    wqt = np.ascontiguousarray(np.asarray(Wq, np.float32).T).astype(NPBF)
    wkt = np.ascontiguousarray(np.asarray(Wk, np.float32).T).astype(NPBF)
    wvt = np.ascontiguousarray(np.asarray(Wv, np.float32).T).astype(NPBF)
    wot = np.ascontiguousarray(np.asarray(Wo, np.float32).T).astype(NPBF)
    bo2 = np.asarray(bo, np.float32).reshape(1, E).astype(NPBF)

    in_maps = []
    for c in range(NCORES):
        b, sh = c // 2, c % 2
        xt = np.ascontiguousarray(X[b].T)  # [E, S]
        if sh == 1:  # rotate so the local query half comes first
            xt = np.concatenate([xt[:, SL:], xt[:, :SL]], axis=1)
        in_maps.append(
            {
                "xt": np.ascontiguousarray(xt.astype(NPBF)),
                "wqt": wqt,
                "wkt": wkt,
                "wvt": wvt,
                "wot": wot,
                "bo": bo2,
            }
        )
    return in_maps


LAST_EXEC_NS = None
LAST_RESULTS = None


def _ensure_ntff_hook_importable():
    """bass_utils imports antenv.axon_hooks when tracing is requested (e.g.
    BASS_TRACE=1 in the environment).  The RL container's antenv stub lacks
    that module; register a no-op fallback so tracing degrades gracefully
    instead of crashing.  If a real antenv.axon_hooks exists, do nothing."""
    import sys
    import types

    try:
        import antenv.axon_hooks  # noqa: F401

        return
    except ImportError:
        pass
    try:
        import antenv

        mod = types.ModuleType("antenv.axon_hooks")
        _hook = [None]
        mod.set_axon_ntff_profile_hook = lambda h: _hook.__setitem__(0, h)
        mod.get_axon_ntff_profile_hook = lambda: _hook[0]
        sys.modules["antenv.axon_hooks"] = mod
        antenv.axon_hooks = mod
        try:
            from trn_agent_boot.trn_boot import _ntff_profile_via_ctypes

            mod.set_axon_ntff_profile_hook(
                _ntff_profile_via_ctypes("/opt/axon/libaxon_pjrt.so")
            )
        except Exception:
            pass
    except Exception:
        pass


def _run(in_maps, trace=False):
    global LAST_EXEC_NS, LAST_RESULTS
    _ensure_ntff_hook_importable()
    if "nc" not in _CACHE:
        _CACHE["nc"] = build()
    res = run_bass_kernel_spmd(
        _CACHE["nc"],
        in_maps,
        core_ids=list(range(NCORES)),
        trace=trace,
    )
    LAST_RESULTS = res
    LAST_EXEC_NS = res.exec_time_ns
    return res


def kernel(X, Wq, Wk, Wv, Wo, bo):
    in_maps = _prep_inputs(X, Wq, Wk, Wv, Wo, bo)
    res = _run(in_maps, trace=bool(int(os.environ.get("KERNEL_TRACE", "0"))))
    out = np.empty((B, S, E), np.float32)
    for c in range(NCORES):
        b, sh = c // 2, c % 2
        out[b, sh * SL : (sh + 1) * SL, :] = res.results[c]["out"]
    return out


# revision 20
# speedup vs baseline: 1.6898x; 1.1392x over previous
"""Multi-headed attention kernel for 8 Trainium2 NeuronCores.

Problem: B=4, S=2048, E=1024, H=16, D=64 (torch-convention Linears, no bias
on q/k/v, bias on output projection).

Sharding: core c handles (batch b = c//2, query half sh = c%2).  Each core
computes Q for its 1024 query rows, K/V for the full 2048 keys of its batch
(duplicated across the pair of cores sharing a batch -- cheaper than any
cross-core collective at these sizes), all 16 heads of attention for its
rows, and the output projection + bias.  Zero collectives.

Layout (feature dim on partitions; scores computed transposed):
  qT[f, q]  = sum_e WqT[e, f] * XT[e, q]          kT[f, s] likewise
  V[s, f]   = sum_e XT[e, s-chunk] * WvT[e, f]    (natural layout)
  scoresT[k, q] = sum_d kT[h*64+d, kc] * qT[h*64+d, q]    (K=64 matmuls,
      head pair packed in complementary PE ROW groups -> concurrent)
  EX = exp(scoresT / 8)            (ACT engine, PSUM -> SBUF bf16)
  ctxT[hh*64+m, q] = sum_k V[k, h*64+m] * EX[k, q]   per head, M=64, the two
      heads packed in complementary PE COLUMN groups -> concurrent (2x over
      an M=65 ones-column formulation)
  den[q] = sum_k EX[k, q]: EX tiles accumulated elementwise in TWO
      independent chains (DVE even kc, GpSimd odd kc; merged at drain), then
      one gpsimd partition_all_reduce per segment (f32 internal);
      normalization = DVE multiply by reciprocal.  Keeping the chains
      engine-local avoids cross-engine semaphore ping-pong that would gate
      the slot rate and let the PE HAM clock-gate re-throttle.
  out[s, e] = sum_f ctxT_norm[f, s-chunk] * WoT[f, e] + bo (bias added on
      DVE from a partition-broadcast bo tile; no PE bias matmuls)

Scheduling: fc-outer over the 8 head pairs, two 512-query segments each.
Projection work for pair fc+1 and V feature-halves is emitted as paired-
weight filler units (two PSUM accumulators sharing each LDWEIGHTS) inside
the 16-key-chunk slot loop, keeping the PE dense while ACT streams exps.
The first query-half's output projection runs as fillers inside the last
pair's second segment; only the second half's runs in the tail.
PSUM: scores 2x[128,1024] + ctx 1x[128,512] + filler 3x[128,512] = 8 banks.
"""

import os

import numpy as np
import ml_dtypes

import concourse.bass as bass
from concourse import bacc
from concourse import bass_isa
import concourse.mybir as mybir
import concourse.tile as tile
from concourse.bass_utils import run_bass_kernel_spmd

B, S, E, H = 4, 2048, 1024, 16
D = E // H  # 64
P = 128
SL = S // 2     # local query rows per core (1024)
NCORES = 8
EC = E // P     # 8 e-chunks
FC = E // P     # 8 feature chunks (head pairs)
SC = S // P     # 16 s-chunks (V natural layout)
KC = S // P     # 16 key chunks (scores partition dim)

F32 = mybir.dt.float32
BF16 = mybir.dt.bfloat16
EXPF = mybir.ActivationFunctionType.Exp
NPBF = ml_dtypes.bfloat16

_CACHE = {}


def build():
    nc = bacc.Bacc(
        "TRN2",
        target_bir_lowering=False,
        debug=False,
        num_devices=NCORES,
    )

    xt_d = nc.dram_tensor("xt", [E, S], BF16, kind="ExternalInput").ap()
    wqt_d = nc.dram_tensor("wqt", [E, E], BF16, kind="ExternalInput").ap()
    wkt_d = nc.dram_tensor("wkt", [E, E], BF16, kind="ExternalInput").ap()
    wvt_d = nc.dram_tensor("wvt", [E, E], BF16, kind="ExternalInput").ap()
    wot_d = nc.dram_tensor("wot", [E, E], BF16, kind="ExternalInput").ap()
    bo_d = nc.dram_tensor("bo", [1, E], BF16, kind="ExternalInput").ap()
    sel2_d = nc.dram_tensor("sel2", [2, P], mybir.dt.float32r,
                            kind="ExternalInput").ap()
    out_d = nc.dram_tensor("out", [SL, E], F32, kind="ExternalOutput").ap()

    with tile.TileContext(nc) as tc:
     with tc.tile_pool(name="persist", bufs=1) as persist:
        DA = D + 1  # head dim + ones column (denominator rides row 64)
        qt_sb = persist.tile([P, FC, SL], BF16, tag="qt")
        kt_sb = persist.tile([P, FC, S], BF16, tag="kt")
        v_sb = persist.tile([P, SC, H * DA], BF16, tag="v")
        vview = v_sb.rearrange("p c (h d) -> p c h d", d=DA)
        nc.vector.memset(vview[:, :, :, D:D + 1], 1.0)
        ctxt_sb = persist.tile([P, FC, SL], BF16, tag="ctxt")
        bo_row = persist.tile([1, E], BF16, tag="bo_row")
        bo_bc = persist.tile([P, E], BF16, tag="bo_bc")
        # sel2: one bcast matmul spreads both heads' denominator rows:
        # out[0:64] <- den_A, out[64:128] <- den_B
        sel2 = persist.tile([2, P], mybir.dt.float32r, tag="sel2")
        nc.sync.dma_start(out=bo_row[:], in_=bo_d[:])
        nc.sync.dma_start(out=sel2[:], in_=sel2_d[:])
        nc.gpsimd.partition_broadcast(bo_bc[:], bo_row[:], channels=P)

        with tc.tile_pool(name="expden", bufs=1) as expden:

            def new_ex():
                return expden.tile([P, 1024], BF16, tag="exp", bufs=8, name="ex")

            def new_densb(tag):
                return expden.tile([1, 512], mybir.dt.float32r, tag=tag,
                                   bufs=2, name=tag)

            def new_rec():
                return expden.tile([P, 512], F32, tag="rec", bufs=2, name="rec")

            # ---------------- loop PSUM pools (manual; PSUM-space LIFO) ----
            # opened after the upfront pool is released, below.

            with tc.tile_pool(name="xw", bufs=1) as xw:
                # ---- input DMAs, ordered for startup latency ----
                w_slices = {}

                def load_wslices(fc):
                    """16 [128,128] lhsT slices (all e-chunks) of WqT/WkT."""
                    qs, ks = [], []
                    for w_dram, lst in ((wqt_d, qs), (wkt_d, ks)):
                        for ec in range(EC):
                            t = xw.tile([P, P], BF16, tag="wqk", bufs=36,
                                        name="wqk")
                            nc.sync.dma_start(
                                out=t[:],
                                in_=w_dram[ec * P:(ec + 1) * P,
                                           fc * P:(fc + 1) * P],
                            )
                            lst.append(t)
                    w_slices[fc] = (qs, ks)

                load_wslices(0)
                xts = []  # xts[ec][half] = [P, 1024]
                for ec in range(EC):
                    row = []
                    for hx in range(2):
                        t = xw.tile([P, 1024], BF16, tag="x", bufs=16,
                                    name="x")
                        nc.sync.dma_start(
                            out=t[:],
                            in_=xt_d[ec * P:(ec + 1) * P,
                                     hx * 1024:(hx + 1) * 1024],
                        )
                        row.append(t)
                    xts.append(row)

                # ---------- filler unit emitters (paired LDWEIGHTS) ----------
                def q_pair(pool, fc):
                    """qt[:, fc, :] (both 512-blocks), wq slice loaded once/ec."""
                    qs = w_slices[fc][0]
                    ps0 = pool.tile([P, 512], F32, tag="pj", name="pj")
                    ps1 = pool.tile([P, 512], F32, tag="pj", name="pj")
                    for ec in range(EC):
                        st, sp = ec == 0, ec == EC - 1
                        nc.tensor.matmul(ps0[:], qs[ec][:],
                                         xts[ec][0][:, 0:512], start=st, stop=sp)
                        nc.tensor.matmul(ps1[:], qs[ec][:],
                                         xts[ec][0][:, 512:1024], start=st, stop=sp)
                    nc.vector.tensor_copy(out=qt_sb[:, fc, 0:512], in_=ps0[:])
                    nc.vector.tensor_copy(out=qt_sb[:, fc, 512:1024], in_=ps1[:])

                def k_pair(pool, fc, j):
                    """kt[:, fc, j*1024:(j+1)*1024], wk slice loaded once/ec."""
                    ks = w_slices[fc][1]
                    ps0 = pool.tile([P, 512], F32, tag="pj", name="pj")
                    ps1 = pool.tile([P, 512], F32, tag="pj", name="pj")
                    for ec in range(EC):
                        st, sp = ec == 0, ec == EC - 1
                        nc.tensor.matmul(ps0[:], ks[ec][:],
                                         xts[ec][j][:, 0:512], start=st, stop=sp)
                        nc.tensor.matmul(ps1[:], ks[ec][:],
                                         xts[ec][j][:, 512:1024], start=st, stop=sp)
                    base = j * 1024
                    nc.vector.tensor_copy(out=kt_sb[:, fc, base:base + 512],
                                          in_=ps0[:])
                    nc.vector.tensor_copy(out=kt_sb[:, fc, base + 512:base + 1024],
                                          in_=ps1[:])

                queues = {fc: [] for fc in range(FC)}
                pending = []  # deferred norm finishers (popped next segment)

                def segment(qv, fc, every_slot=False, pop_from=1):
                    hA, hB = 2 * fc, 2 * fc + 1
                    q = queues[fc]
                    ctx_a = psum_cx.tile([DA, 512], F32, tag="ctx", name="ctx")
                    ctx_b = psum_cx.tile([DA, 512], F32, tag="ctx", name="ctx")
                    for kc in range(KC):
                        sc_ps = psum_sc.tile([P, 1024], F32, tag="sc",
                                             name="sc")
                        for hh in (0, 1):
                            po = hh * D
                            nc.tensor.matmul(
                                sc_ps[:, hh * 512:hh * 512 + 512],
                                kt_sb[po:po + D, fc, kc * P:(kc + 1) * P],
                                qt_sb[po:po + D, fc,
                                      qv * 512:qv * 512 + 512],
                                start=True, stop=True)
                        ex = new_ex()
                        nc.scalar.activation(ex[:], sc_ps[:], EXPF,
                                             scale=0.125)
                        for cps, hh, h in ((ctx_a, 0, hA), (ctx_b, 1, hB)):
                            nc.tensor.matmul(
                                cps[0:DA, :],
                                v_sb[:, kc, h * DA:(h + 1) * DA],
                                ex[:, hh * 512:hh * 512 + 512],
                                start=(kc == 0), stop=(kc == KC - 1))
                        if kc == 2 and pending:
                            pending.pop(0)()
                        if q and (every_slot or (kc % 2 and kc >= pop_from)):
                            q.pop(0)()
                    # drain: ctx + denominator rows -> sbuf (frees psum)
                    dst = ctxt_sb[:, fc, qv * 512:qv * 512 + 512]
                    den_a = new_densb("dena")
                    den_b = new_densb("denb")
                    nc.vector.tensor_copy(out=dst[0:D, :], in_=ctx_a[0:D, :])
                    nc.vector.tensor_copy(out=dst[D:P, :], in_=ctx_b[0:D, :])
                    nc.vector.tensor_copy(out=den_a[:], in_=ctx_a[D:DA, :])
                    nc.vector.tensor_copy(out=den_b[:], in_=ctx_b[D:DA, :])

                    def _finish(den_a=den_a, den_b=den_b, dst=dst):
                        # broadcast each denominator row across 64 partitions
                        # (K=1 matmul, shared ones-row lhsT), reciprocal,
                        # then normalize; recip can't shift partitions, so
                        # head B goes through rtmp + cross-partition copy.
                        po = psum_pj.tile([P, 512], F32, tag="pj", name="pj")
                        po2 = psum_pj.tile([P, 512], F32, tag="pj", name="pj")
                        nc.tensor.matmul(po[0:D, :], sel2[0:1, 0:D], den_a[:],
                                         start=True, stop=True)
                        nc.tensor.matmul(po2[0:D, :], sel2[0:1, 0:D], den_b[:],
                                         start=True, stop=True)
                        rec = new_rec()
                        rtmp = expden.tile([D, 512], F32, tag="rtmp", bufs=2,
                                           name="rtmp")
                        nc.vector.reciprocal_approx_fast(out=rec[0:D, :],
                                                         in_=po[0:D, :])
                        nc.vector.reciprocal_approx_fast(out=rtmp[:],
                                                         in_=po2[0:D, :])
                        nc.vector.tensor_copy(out=rec[D:P, :], in_=rtmp[:])
                        nc.vector.tensor_mul(out=dst[0:D, :], in0=dst[0:D, :],
                                             in1=rec[0:D, :])
                        nc.vector.tensor_mul(out=dst[D:P, :], in0=dst[D:P, :],
                                             in1=rec[D:P, :])

                    pending.append(_finish)

                with tc.tile_pool(name="wvp", bufs=1) as wvp:
                    wv = []
                    for ec in range(EC):
                        t = wvp.tile([P, E], BF16, tag="wv", bufs=8, name="wv")
                        nc.sync.dma_start(out=t[:],
                                          in_=wvt_d[ec * P:(ec + 1) * P, :])
                        wv.append(t)

                    def v_unit(pool, sc, fb, bufs=None):
                        """v_sb[:, sc, fb*512:...]: natural-layout V chunk."""
                        ps = pool.tile([P, 512], F32, tag="pj", bufs=bufs,
                                       name="pj")
                        for ec in range(EC):
                            nc.tensor.matmul(
                                ps[:],
                                xts[ec][sc // 8][:, (sc % 8) * P:(sc % 8 + 1) * P],
                                wv[ec][:, fb * 512:(fb + 1) * 512],
                                start=(ec == 0), stop=(ec == EC - 1),
                            )
                        vv = v_sb[:, sc, :].rearrange("p (h d) -> p h d", d=DA)
                        nc.vector.tensor_copy(
                            out=vv[:, fb * 8:(fb + 1) * 8, 0:D],
                            in_=ps.rearrange("p (h d) -> p h d", d=D))

                    # ---------------- upfront: QK(fc0) + V(sc0-5, fb0) -------
                    psum_u = tc.alloc_tile_pool(name="psum_u", bufs=6,
                                                space="PSUM")
                    psq = [psum_u.tile([P, 512], F32, tag="u", name="u")
                           for _ in range(2)]
                    psk = [psum_u.tile([P, 512], F32, tag="u", name="u")
                           for _ in range(4)]
                    qs0, ks0 = w_slices[0]
                    for ec in range(EC):
                        st, sp = ec == 0, ec == EC - 1
                        nc.tensor.matmul(psq[0][:], qs0[ec][:],
                                         xts[ec][0][:, 0:512], start=st, stop=sp)
                        nc.tensor.matmul(psq[1][:], qs0[ec][:],
                                         xts[ec][0][:, 512:1024], start=st, stop=sp)
                        for kb in range(4):
                            nc.tensor.matmul(
                                psk[kb][:], ks0[ec][:],
                                xts[ec][kb // 2][:, (kb % 2) * 512:(kb % 2) * 512 + 512],
                                start=st, stop=sp)
                    nc.vector.tensor_copy(out=qt_sb[:, 0, 0:512], in_=psq[0][:])
                    nc.vector.tensor_copy(out=qt_sb[:, 0, 512:1024], in_=psq[1][:])
                    for kb in range(4):
                        nc.vector.tensor_copy(
                            out=kt_sb[:, 0, kb * 512:(kb + 1) * 512],
                            in_=psk[kb][:])
                    for sc in range(6):
                        v_unit(psum_u, sc, 0, bufs=2)
                    psum_u.release()

                    # ---------------- pair loop (fc0-4 with wv live) --------
                    psum_sc = tc.alloc_tile_pool(name="psum_sc", bufs=2,
                                                 space="PSUM")
                    psum_cx = tc.alloc_tile_pool(name="psum_cx", bufs=2,
                                                 space="PSUM")
                    psum_pj = tc.alloc_tile_pool(name="psum_pj", bufs=2,
                                                 space="PSUM")

                    queues[0] = (
                        [lambda sc=sc: v_unit(psum_pj, sc, 0)
                         for sc in range(6, 16)]
                    )
                    vf1 = [lambda sc=sc: v_unit(psum_pj, sc, 1)
                           for sc in range(16)]
                    vf1_share = {1: vf1[0:2], 2: vf1[2:7], 3: vf1[7:12],
                                 4: vf1[12:16]}
                    for fc in range(FC - 1):
                        nfc = fc + 1
                        queues[fc] += [
                            lambda f=nfc: q_pair(psum_pj, f),
                            lambda f=nfc: k_pair(psum_pj, f, 0),
                            lambda f=nfc: k_pair(psum_pj, f, 1),
                        ]
                    for fc, units in vf1_share.items():
                        queues[fc] += units

                    for fc in range(5):
                        if fc + 1 < FC:
                            load_wslices(fc + 1)
                        segment(0, fc, every_slot=(fc == 0))
                        segment(1, fc, every_slot=(fc == 0))
                        while queues[fc]:
                            queues[fc].pop(0)()
                # wv released; fc5-6 (x + wqk still live)
                for fc in (5, 6):
                    if fc + 1 < FC:
                        load_wslices(fc + 1)
                    segment(0, fc)
                    segment(1, fc)
                    while queues[fc]:
                        queues[fc].pop(0)()

            # xw released: fc7 + output projection
            with (
                tc.tile_pool(name="wo", bufs=1) as wopool,
                tc.tile_pool(name="outp", bufs=3) as outpool,
            ):
                wot_sb = wopool.tile([P, FC, E], BF16, tag="wot")
                for fcc in range(FC):
                    nc.sync.dma_start(
                        out=wot_sb[:, fcc, :],
                        in_=wot_d[fcc * P:(fcc + 1) * P, :])

                ots = {}

                def o_half(sc, eb):
                    po = psum_pj.tile([P, 512], F32, tag="pj", name="pj")
                    for fcc in range(FC):
                        st, sp = fcc == 0, fcc == FC - 1
                        nc.tensor.matmul(po[:],
                                         ctxt_sb[:, fcc, sc * P:(sc + 1) * P],
                                         wot_sb[:, fcc,
                                                eb * 512:(eb + 1) * 512],
                                         start=st, stop=sp)
                    if eb == 0:
                        ots[sc] = outpool.tile([P, E], F32, tag="out",
                                               name="out")
                    ot = ots[sc]
                    nc.vector.tensor_add(out=ot[:, eb * 512:(eb + 1) * 512],
                                         in0=po[:],
                                         in1=bo_bc[:, eb * 512:(eb + 1) * 512])
                    if eb == 1:
                        nc.sync.dma_start(out=out_d[sc * P:(sc + 1) * P, :],
                                          in_=ot[:])

                segment(0, 7)
                queues[7] = [lambda sc=sc, eb=eb: o_half(sc, eb)
                             for sc in range(4) for eb in range(2)]
                segment(1, 7, pop_from=3)
                while pending:
                    pending.pop(0)()
                while queues[7]:
                    queues[7].pop(0)()
                for sc in range(4, SL // P):
                    o_half(sc, 0)
                    o_half(sc, 1)

                psum_pj.release()
                psum_cx.release()
                psum_sc.release()

    nc.compile()
    return nc


def _prep_inputs(X, Wq, Wk, Wv, Wo, bo):
    X = np.asarray(X, dtype=np.float32)
    wqt = np.ascontiguousarray(np.asarray(Wq, np.float32).T).astype(NPBF)
    wkt = np.ascontiguousarray(np.asarray(Wk, np.float32).T).astype(NPBF)
    wvt = np.ascontiguousarray(np.asarray(Wv, np.float32).T).astype(NPBF)
    wot = np.ascontiguousarray(np.asarray(Wo, np.float32).T).astype(NPBF)
    bo2 = np.asarray(bo, np.float32).reshape(1, E).astype(NPBF)
    sel2 = np.zeros((2, 128), np.float32)
    sel2[0, 0:D] = 1.0
    sel2[1, D:128] = 1.0

    in_maps = []
    for c in range(NCORES):
        b, sh = c // 2, c % 2
        xt = np.ascontiguousarray(X[b].T)  # [E, S]
        if sh == 1:  # rotate so the local query half comes first
            xt = np.concatenate([xt[:, SL:], xt[:, :SL]], axis=1)
        in_maps.append(
            {
                "xt": np.ascontiguousarray(xt.astype(NPBF)),
                "wqt": wqt,
                "wkt": wkt,
                "wvt": wvt,
                "wot": wot,
                "bo": bo2,
                "sel2": sel2,
            }
        )
    return in_maps


LAST_EXEC_NS = None
LAST_RESULTS = None


def _ensure_ntff_hook_importable():
    """bass_utils imports antenv.axon_hooks when tracing is requested (e.g.
    BASS_TRACE=1 in the environment).  The RL container's antenv stub lacks
    that module; register a no-op fallback so tracing degrades gracefully
    instead of crashing.  If a real antenv.axon_hooks exists, do nothing."""
    import sys
    import types

    try:
        import antenv.axon_hooks  # noqa: F401

        return
    except ImportError:
        pass
    try:
        import antenv

        mod = types.ModuleType("antenv.axon_hooks")
        _hook = [None]
        mod.set_axon_ntff_profile_hook = lambda h: _hook.__setitem__(0, h)
        mod.get_axon_ntff_profile_hook = lambda: _hook[0]
        sys.modules["antenv.axon_hooks"] = mod
        antenv.axon_hooks = mod
        try:
            from trn_agent_boot.trn_boot import _ntff_profile_via_ctypes

            mod.set_axon_ntff_profile_hook(
                _ntff_profile_via_ctypes("/opt/axon/libaxon_pjrt.so")
            )
        except Exception:
            pass
    except Exception:
        pass


def _run(in_maps, trace=False):
    global LAST_EXEC_NS, LAST_RESULTS
    _ensure_ntff_hook_importable()
    if "nc" not in _CACHE:
        _CACHE["nc"] = build()
    res = run_bass_kernel_spmd(
        _CACHE["nc"],
        in_maps,
        core_ids=list(range(NCORES)),
        trace=trace,
    )
    LAST_RESULTS = res
    LAST_EXEC_NS = res.exec_time_ns
    return res


def kernel(X, Wq, Wk, Wv, Wo, bo):
    in_maps = _prep_inputs(X, Wq, Wk, Wv, Wo, bo)
    res = _run(in_maps, trace=bool(int(os.environ.get("KERNEL_TRACE", "0"))))
    out = np.empty((B, S, E), np.float32)
    for c in range(NCORES):
        b, sh = c // 2, c % 2
        out[b, sh * SL : (sh + 1) * SL, :] = res.results[c]["out"]
    return out
